# revision 2
# baseline (speedup 1.0000x reference)
"""Trainium2 Bass kernel v3: per-channel 256-bin normalized histogram.

Input: full inputs [64, 512, 512, 3] float32 in [0, 1).
Output: [256, 3] float32 - per-channel histogram normalized to sum 1.

8 cores data-parallel over batch. Per core 6,291,456 elements laid out
[128 partitions, 3 channels, 16384 pixels] (host pre-transposed).

All pixels go through a PE nibble-joint route:
  Per chunk (FC=1024 pixel columns per channel):
    idx = RNE(x*256 - 0.5) int16 (1 DVE pass, exact floor).
    One-hot tiles in packed layout [P, NMM, 16, FPACK=8]:
      hi ge rows  G[.., j, ..] = [idx >= 16j]: rows in ACT_ROWS built by
        ScalarE as +-1 Sign(256x - 16j + delta) from raw x; the rest
        (incl. row 0 = all ones) by VectorE is_ge from idx.
      lo one-hot  L[.., l, ..] = [(idx & 15) == l]: VectorE (and + 16 eq).
    PE: per f-block m: lhsT = G[:, m] (M=128 contiguous weight cols ->
      auto Fast-Weight-Load), rhs = L[:, m] (N=128), accumulated into
      psum_c[128, 128]; entries ps[8j+d, 8l+d] (same d) are the wanted
      joint partial sums, off-diagonal d pairs are harmless garbage.
      One accumulation chain per channel.
  Host: Cge[j,l] = sum_d ps[8j+d, 8l+d]; +-1 rows decoded via row 0;
  counts J[j,l] = Cge[j]-Cge[j+1]; exact integers in fp32.

Exactness: integer counts < 2^24 in fp32 PSUM; only ACT-built rows have a
delta=2^-12 threshold fuzz (a few elements per boundary), far inside the
2e-2 tolerance.
"""

import os

import numpy as np

import concourse.bacc as bacc
import concourse.mybir as mybir
from concourse.bass_utils import run_bass_kernel_spmd
from concourse.tile import TileContext

# Problem constants
B, H, W, C = 64, 512, 512, 3
NBINS = 256
NCORES = 8
P = 128
PIXROW = 16384                    # pixels per channel per partition

FC = 1024                         # chunk size (pixel cols per channel)
NCH = PIXROW // FC                # 16 chunks
FPACK = 8                         # f-positions packed per matmul
NMM = FC // FPACK                 # matmuls per chunk-channel

# hi rows built by ScalarE (as +-1 signs) instead of VectorE; row 0 must
# stay on DVE (all-ones reference row for +-1 decode).
ACT_ROWS = tuple(range(7, 16))

DELTA = 2.0 ** -12
AL = mybir.AluOpType
AF = mybir.ActivationFunctionType

_CACHE: dict = {}


def _build_module():
    nc = bacc.Bacc("TRN2", target_bir_lowering=False, debug=False,
                   num_devices=NCORES)

    x_ext = nc.declare_dram_parameter("x", [P, C * PIXROW], mybir.dt.float32,
                                      isOutput=False)
    bias_ext = nc.declare_dram_parameter("bias_tab", [P, 16],
                                         mybir.dt.float32, isOutput=False)
    pj_ext = nc.declare_dram_parameter("pjoint", [P, 128 * C],
                                       mybir.dt.float32, isOutput=True)

    CF = C * FC

    with TileContext(nc) as tc:
        with (tc.tile_pool(name="persist", bufs=1) as pp,
              tc.tile_pool(name="stage", bufs=2) as stp,
              tc.tile_pool(name="oh", bufs=2) as ohp,
              tc.tile_pool(name="psum", bufs=1, space="PSUM") as psp):

            bias_tab = pp.tile([P, 16], mybir.dt.float32, tag="bias")
            outp = pp.tile([P, 128 * C], mybir.dt.float32, tag="outp")
            nc.sync.dma_start(out=bias_tab[:], in_=bias_ext.ap())

            ps = []
            for c in range(C):
                psc = psp.tile([P, 128], mybir.dt.float32, tag=f"ps{c}",
                               name=f"ps{c}")
                ps.append(psc)

            # Row 0 of the hi tile is the all-ones ge-0 row: write it once
            # into both physical buffers of the rotating pool and skip the
            # per-chunk pass (the loop's tile() calls cycle these 2 slots in
            # the same order, so the data persists untouched).
            for slot in range(2):
                hi_pre = ohp.tile([P, NMM, 16, FPACK], mybir.dt.bfloat16,
                                  tag="hi", name=f"hi_pre{slot}")
                nc.gpsimd.memset(hi_pre[:, :, 0, :], 1.0)

            for k in range(NCH):
                xs = stp.tile([P, CF], mybir.dt.float32, tag="xs")
                idx = stp.tile([P, CF], mybir.dt.int16, tag="idx")
                for c in range(C):
                    nc.sync.dma_start(
                        out=xs[:, c * FC:(c + 1) * FC],
                        in_=x_ext.ap()[:, c * PIXROW + k * FC:
                                       c * PIXROW + (k + 1) * FC])
                # idx = RNE(x*256 - 0.5) = floor(x*256)  (int16 cast is RNE)
                nc.vector.tensor_scalar(idx[:], xs[:], 256.0, -0.5,
                                        AL.mult, AL.add)
                for c in range(C):
                    hi_t = ohp.tile([P, NMM, 16, FPACK], mybir.dt.bfloat16,
                                    tag="hi")
                    lo_t = ohp.tile([P, NMM, 16, FPACK], mybir.dt.bfloat16,
                                    tag="lo")
                    lo16 = ohp.tile([P, FC], mybir.dt.int16, tag="lo16")
                    src = idx[:, c * FC:(c + 1) * FC]
                    for j in range(1, 16):
                        if j in ACT_ROWS:
                            nc.scalar.activation(
                                hi_t[:, :, j, :], xs[:, c * FC:(c + 1) * FC],
                                AF.Sign, bias=bias_tab[:, j:j + 1],
                                scale=256.0)
                        else:
                            nc.vector.tensor_scalar(
                                hi_t[:, :, j, :], src, float(16 * j), None,
                                AL.is_ge)
                    nc.vector.tensor_scalar(lo16[:], src, 15, None,
                                            AL.bitwise_and)
                    for l in range(16):
                        nc.vector.tensor_scalar(lo_t[:, :, l, :], lo16[:],
                                                float(l), None, AL.is_equal)
                    for m in range(NMM):
                        nc.tensor.matmul(
                            ps[c][:], hi_t[:, m], lo_t[:, m],
                            start=(k == 0 and m == 0),
                            stop=(k == NCH - 1 and m == NMM - 1),
                            skip_group_check=True)

            for c in range(C):
                nc.vector.tensor_copy(outp[:, 128 * c:128 * (c + 1)],
                                      ps[c][:])
            nc.sync.dma_start(out=pj_ext.ap(), in_=outp[:])

    nc.finalize()
    return nc


def _get_module():
    if "nc" not in _CACHE:
        _CACHE["nc"] = _build_module()
    return _CACHE["nc"]


def _decode_counts(results):
    """Exact per-channel counts [C, NBINS] (float64) from the 8 cores."""
    counts = np.zeros((C, NBINS), dtype=np.float64)
    d = np.arange(FPACK)
    for r in results:
        pj = r["pjoint"].astype(np.float64)     # [128, 128*C]
        for c in range(C):
            psc = pj[:, 128 * c:128 * (c + 1)]  # [8j+d, 8l+d']
            # Cge[j, l] = sum_d psc[8j+d, 8l+d]
            Cge = psc.reshape(16, FPACK, 16, FPACK)[:, d, :, d].sum(axis=0)
            # +-1 rows -> ge counts using row 0 (= block count)
            for j in ACT_ROWS:
                Cge[j] = (Cge[j] + Cge[0]) / 2.0
            J = np.empty((16, 16))
            J[:15] = Cge[:15] - Cge[1:]
            J[15] = Cge[15]
            counts[c] += J.reshape(NBINS)
    return counts


def run(x: np.ndarray, trace: bool = False):
    nc = _get_module()

    x = np.ascontiguousarray(x, dtype=np.float32)
    assert x.shape == (B, H, W, C)
    shards = x.reshape(NCORES, P, PIXROW, C).transpose(0, 1, 3, 2)
    shards = np.ascontiguousarray(shards).reshape(NCORES, P, C * PIXROW)

    bias_tab = np.tile(
        (DELTA - 16.0 * np.arange(16, dtype=np.float32))[None, :], (P, 1))
    in_maps = [{"x": shards[i], "bias_tab": bias_tab} for i in range(NCORES)]

    res = run_bass_kernel_spmd(nc, in_maps, list(range(NCORES)), trace=trace)

    counts = _decode_counts(res.results)
    counts32 = counts.astype(np.float32)
    sums = counts32.sum(axis=1, keepdims=True, dtype=np.float32)
    hist = counts32 / sums
    return np.ascontiguousarray(hist.T), res


def kernel(**inputs) -> np.ndarray:
    out, _ = run(inputs["inputs"],
                 trace=bool(os.environ.get("KERNEL_TRACE")))
    return out
